# revision 1
# baseline (speedup 1.0000x reference)
"""Trainium2 Bass kernel for nn_MixtureOfExperts (B=524288, IN=59, E=4, H=64).

Strategy (pure data parallel over 8 cores, 65536 rows each):
 - Host folds BN into weights (scale into W, shift into per-feature bias),
   collapses expert head w3@wp -> wep (H->1), and pre-transposes x into a
   feature-on-partition layout so no on-chip transposes are needed.
 - On chip, everything is feature-major [feat, batch] with batch tiles of 512
   on the matmul moving dim.
 - Stage 1 + gating hidden run in float32r (full PE rate; ISA requires dst
   partition 0, so their outputs are full [128,*] tiles / zero-col-padded
   accumulations).  Stage 2 / preds / logits run in bf16 (dst partition can
   be 32-aligned, enabling strip packing + 4-way quadrant concurrency).
 - x is packed [128, S] with two independent 64-feature batch-halves on
   partition halves, so stage-1 matmuls for consecutive tiles land on
   disjoint PE row strips and overlap on the array.
 - Tiny outputs (gate hidden, logits, per-expert preds) are packed into full
   [128, 512] PSUM tiles via strips / zero-padded lhsT columns so the
   PSUM->SBUF hops always run with all 128 lanes busy.
 - softmax-weighted combine: pred = sum_e exp(l_e)*(p_e+b_e) / sum_e exp(l_e)
   (no max-subtraction needed; logits are O(1)).
"""

import numpy as np
import ml_dtypes

import concourse.bass as bass
import concourse.mybir as mybir
import concourse.tile as tile
from concourse import bacc
from concourse.bass_utils import run_bass_kernel_spmd

F32 = mybir.dt.float32
F32R = mybir.dt.float32r
BF16 = mybir.dt.bfloat16
AF = mybir.ActivationFunctionType
ALU = mybir.AluOpType

B, IN, E, H, EMB, GH = 524288, 59, 4, 64, 32, 32
EPS = 1e-5
NCORES = 8
BC = B // NCORES          # 65536 rows per core
S = 8192                  # rows per batch-half per superstep
SUP = BC // (2 * S)       # 4 supersteps
NT = (2 * S) // 512       # 32 tiles per superstep
BT = 512
W_F = 768 + 32 + 8        # f32r wts width: w1(256)+gate(512) | gsum | bias
W_B = 128 + 512 + 1024    # bf16 wts width: w2 | wep | gw2

_CACHE = {}


def _build():
    nc = bacc.Bacc(trn_type="TRN2")
    x_d = nc.dram_tensor("x", (SUP, 128, S), F32R, kind="ExternalInput")
    wts_d = nc.dram_tensor("wts", (128, W_F), F32R, kind="ExternalInput")
    wtsb_d = nc.dram_tensor("wtsb", (128, W_B), BF16, kind="ExternalInput")
    out_d = nc.dram_tensor("out", (SUP, NT, BT), F32, kind="ExternalOutput")

    with tile.TileContext(nc) as tc:
        with (
            tc.tile_pool(name="consts", bufs=1) as consts,
            tc.tile_pool(name="xp", bufs=2) as xp,
            tc.tile_pool(name="hs", bufs=2) as hs,
            tc.tile_pool(name="gts", bufs=2) as gts,
            tc.tile_pool(name="tails", bufs=2) as tails,
            tc.tile_pool(name="ph1a", bufs=1, space="PSUM") as ph1a,
            tc.tile_pool(name="ph1b", bufs=1, space="PSUM") as ph1b,
            tc.tile_pool(name="ph2a", bufs=1, space="PSUM") as ph2a,
            tc.tile_pool(name="ph2b", bufs=1, space="PSUM") as ph2b,
            tc.tile_pool(name="pga", bufs=1, space="PSUM") as pga,
            tc.tile_pool(name="pgb", bufs=1, space="PSUM") as pgb,
            tc.tile_pool(name="pl", bufs=1, space="PSUM") as pl,
            tc.tile_pool(name="pp", bufs=1, space="PSUM") as pp,
        ):
            wts_sb = consts.tile([128, W_F], F32R)
            nc.sync.dma_start(out=wts_sb, in_=wts_d[:, :])
            wtsb_sb = consts.tile([128, W_B], BF16)
            nc.sync.dma_start(out=wtsb_sb, in_=wtsb_d[:, :])
            w1_sb = wts_sb[:, 0:768]          # 0:256 experts, 256:768 gate(4x128)
            gs_sb = wts_sb[:, 768:800]
            bias_sb = wts_sb[:, 800:808].bitcast(F32)
            w2_sb = wtsb_sb[:, 0:128]
            wep_sb = wtsb_sb[:, 128:640].rearrange(
                "p (h j m) -> p h j m", h=2, j=8)
            gw2_sb = wtsb_sb[:, 640:1664].rearrange(
                "p (g j m) -> p g j m", g=4, j=8)
            c2a = bias_sb[:, 0:1]
            c2b = bias_sb[:, 1:2]
            gb2t = bias_sb[:, 2:3]
            bept = bias_sb[:, 3:4]

            for k in range(SUP):
                x_sb = xp.tile([128, S], F32R, tag="x")
                for ch in range(4):
                    cw = S // 4
                    nc.sync.dma_start(
                        out=x_sb[:, ch * cw : (ch + 1) * cw],
                        in_=x_d[k][:, ch * cw : (ch + 1) * cw])

                l_ps = pl.tile([128, BT], F32, tag="l")
                p_ps = pp.tile([128, BT], F32, tag="p")

                for q in range(4):            # group = pairs 4q..4q+3
                    # ---- gating: per-half accumulators (an f32r matmul
                    # group must keep one row base; mixing 0/64 into the
                    # same PSUM tile crashes the device).  4 zero-col-padded
                    # M=128 lhsT slots pack 4 tiles per [128,512] bank.
                    ga_ps = pga.tile([128, BT], F32, tag="ga")
                    gb_ps = pgb.tile([128, BT], F32, tag="gb")
                    for gi in range(4):
                        cg = (4 * q + gi) * BT
                        lt = w1_sb[:, 256 + 128 * gi : 384 + 128 * gi]
                        nc.tensor.matmul(
                            out=ga_ps,
                            lhsT=lt[0:64, :],
                            rhs=x_sb[0:64, cg : cg + BT],
                            start=(gi == 0), stop=(gi == 3),
                            skip_group_check=True,
                        )
                        nc.tensor.matmul(
                            out=gb_ps,
                            lhsT=lt[64:128, :],
                            rhs=x_sb[64:128, cg : cg + BT],
                            start=(gi == 0), stop=(gi == 3),
                            skip_group_check=True,
                        )
                    g1a_sb = gts.tile([128, BT], BF16, tag="g1a")
                    nc.scalar.activation(g1a_sb, ga_ps, AF.Relu)
                    g1b_sb = gts.tile([128, BT], BF16, tag="g1b")
                    nc.scalar.activation(g1b_sb, gb_ps, AF.Relu)

                    for pi in range(4):       # pair inside group
                        pr = 4 * q + pi
                        c0 = pr * BT
                        for half in (0, 1):
                            t = pr + 16 * half
                            base = 64 * half
                            strip = t // 8
                            j = t % 8
                            gslot = pi
                            g1_sb = g1a_sb if half == 0 else g1b_sb
                            xs = x_sb[base : base + 64, c0 : c0 + BT]

                            # ---- stage 1 (f32r, dst 0, M=128).  Biases are
                            # folded into the matmul via the ones-row of x
                            # (row 59 carries c1/gb1 in the weights).
                            h1a_ps = ph1a.tile([128, BT], F32, tag="h1a")
                            nc.tensor.matmul(
                                out=h1a_ps,
                                lhsT=w1_sb[base : base + 64, 0:128],
                                rhs=xs, start=True, stop=True,
                            )
                            h1b_ps = ph1b.tile([128, BT], F32, tag="h1b")
                            nc.tensor.matmul(
                                out=h1b_ps,
                                lhsT=w1_sb[base : base + 64, 128:256],
                                rhs=xs, start=True, stop=True,
                            )
                            h1a_sb = hs.tile([128, BT], BF16, tag="h1as")
                            nc.scalar.activation(h1a_sb, h1a_ps, AF.Relu)
                            h1b_sb = hs.tile([128, BT], BF16, tag="h1bs")
                            nc.vector.tensor_scalar(
                                h1b_sb, h1b_ps, 0.0, None, ALU.max)

                            # ---- stage 2 (bf16): 4 concurrent quadrants
                            h2a_ps = ph2a.tile([128, BT], F32, tag="h2a")
                            h2b_ps = ph2b.tile([128, BT], F32, tag="h2b")
                            nc.tensor.matmul(   # e0
                                out=h2a_ps[0:64, :], lhsT=w2_sb[0:64, 0:64],
                                rhs=h1a_sb[0:64, :], start=True, stop=True)
                            nc.tensor.matmul(   # e1
                                out=h2a_ps[64:128, :],
                                lhsT=w2_sb[64:128, 0:64],
                                rhs=h1a_sb[64:128, :], start=True, stop=True)
                            nc.tensor.matmul(   # e2 -> h2b[64:]
                                out=h2b_ps[64:128, :],
                                lhsT=w2_sb[0:64, 64:128],
                                rhs=h1b_sb[0:64, :], start=True, stop=True)
                            nc.tensor.matmul(   # e3 -> h2b[:64]
                                out=h2b_ps[0:64, :],
                                lhsT=w2_sb[64:128, 64:128],
                                rhs=h1b_sb[64:128, :], start=True, stop=True)
                            h2a_sb = hs.tile([128, BT], BF16, tag="h2as")
                            nc.scalar.activation(
                                h2a_sb, h2a_ps, AF.Relu, bias=c2a)
                            h2b_sb = hs.tile([128, BT], BF16, tag="h2bs")
                            nc.vector.tensor_scalar(
                                h2b_sb, h2b_ps, c2b, 0.0, ALU.add, ALU.max)

                            # ---- stage 3 (bf16): preds into p_ps strip
                            nc.tensor.matmul(
                                out=p_ps[32 * strip : 32 * strip + 32, :],
                                lhsT=wep_sb[:, 0, j, :],
                                rhs=h2a_sb,
                                start=(j == 0), stop=False,
                                skip_group_check=True,
                                tile_position=(0, 32 * strip),
                            )
                            nc.tensor.matmul(
                                out=p_ps[32 * strip : 32 * strip + 32, :],
                                lhsT=wep_sb[:, 1, j, :],
                                rhs=h2b_sb,
                                start=False, stop=(j == 7),
                                skip_group_check=True,
                                tile_position=(0, 32 * strip),
                            )

                            # ---- logits (bf16) into l_ps strip.  K=128
                            # with zero rows outside this gslot's strip so
                            # every mm in the accumulation group keeps row
                            # base 0 (mixed row bases crash the device).
                            nc.tensor.matmul(
                                out=l_ps[32 * strip : 32 * strip + 32, :],
                                lhsT=gw2_sb[:, gslot, j, :],
                                rhs=g1_sb,
                                start=(j == 0), stop=(j == 7),
                                skip_group_check=True,
                                tile_position=(0, 32 * strip),
                            )

                # ---- superstep tail (16384 rows), all full-width ops
                expl_sb = tails.tile([128, BT], F32R, tag="expl")
                nc.scalar.activation(expl_sb, l_ps, AF.Exp, bias=gb2t)
                pb_sb = tails.tile([128, BT], F32R, tag="pb")
                nc.vector.tensor_scalar(pb_sb, p_ps, bept, None, ALU.add)
                w_sb = tails.tile([128, BT], F32R, tag="wsb")
                nc.vector.tensor_mul(w_sb, pb_sb, expl_sb)

                num_ps = pl.tile([32, BT], F32, tag="l")
                nc.tensor.matmul(
                    out=num_ps, lhsT=gs_sb, rhs=w_sb, start=True, stop=True)
                den_ps = pp.tile([32, BT], F32, tag="p")
                nc.tensor.matmul(
                    out=den_ps, lhsT=gs_sb, rhs=expl_sb, start=True,
                    stop=True)
                denr_sb = tails.tile([32, BT], F32, tag="denr")
                out_sb = tails.tile([32, BT], F32, tag="outs")
                nc.vector.reciprocal(denr_sb, den_ps)
                nc.vector.tensor_mul(out_sb, num_ps, denr_sb)
                nc.sync.dma_start(out=out_d[k], in_=out_sb)

    if not nc.is_finalized():
        nc.finalize()
    return nc


def _pack_host(w1, b1, bn1_g, bn1_b, bn1_m, bn1_v, w2, b2, bn2_g, bn2_b,
               bn2_m, bn2_v, w3, b3, wp, bp, gw1, gb1, gw2, gb2):
    f = np.float32
    s1 = (bn1_g / np.sqrt(bn1_v + EPS)).astype(f)              # (E,H)
    w1e = (w1 * s1[:, None, :]).astype(f)                       # (E,IN,H)
    c1 = ((b1 - bn1_m) * s1 + bn1_b).astype(f)                  # (E,H)
    s2 = (bn2_g / np.sqrt(bn2_v + EPS)).astype(f)
    w2e = (w2 * s2[:, None, :]).astype(f)                       # (E,H,H)
    c2 = ((b2 - bn2_m) * s2 + bn2_b).astype(f)                  # (E,H)
    wep = np.einsum("ehm,em->eh", w3, wp).astype(f)             # (E,H)
    bep = (np.einsum("em,em->e", b3, wp) + bp).astype(f)        # (E,)

    # ---- f32r block: w1 experts (256) + gate 4 slots (4x128) + gsum + bias
    w1p = np.zeros((128, 768), f)
    half = np.zeros((64, 768), f)
    half[:IN, 0:64] = w1e[0]
    half[:IN, 64:128] = w1e[1]
    half[:IN, 128:192] = w1e[2]
    half[:IN, 192:256] = w1e[3]
    half[IN, 0:256] = np.concatenate([c1[0], c1[1], c1[2], c1[3]])
    for gi in range(4):
        lo = 256 + 128 * gi + 32 * gi
        half[:IN, lo : lo + 32] = gw1
        half[IN, lo : lo + 32] = gb1
    w1p[0:64] = half
    w1p[64:128] = half

    gsump = np.zeros((128, 32), f)
    for p in range(128):
        gsump[p, 8 * (p // 32) + (p % 32) // 4] = 1.0

    biasp = np.zeros((128, 8), f)
    biasp[:, 0] = np.concatenate([c2[0], c2[1]])
    biasp[:, 1] = np.concatenate([c2[3], c2[2]])   # h2b = [e3; e2]
    biasp[:, 2] = np.tile(gb2, 32)
    biasp[:, 3] = np.tile(bep, 32)

    wts = np.concatenate([w1p, gsump, biasp], axis=1)
    assert wts.shape == (128, W_F), wts.shape

    # ---- bf16 block: w2 quadrants + wep slots + gw2 slots
    w2p = np.zeros((128, 128), f)
    w2p[0:64, 0:64] = w2e[0]
    w2p[64:128, 0:64] = w2e[1]
    w2p[0:64, 64:128] = w2e[2]
    w2p[64:128, 64:128] = w2e[3]

    wepp = np.zeros((128, 2, 8, 32), f)
    for j in range(8):
        wepp[0:64, 0, j, 4 * j + 0] = wep[0]
        wepp[64:128, 0, j, 4 * j + 1] = wep[1]
        wepp[64:128, 1, j, 4 * j + 2] = wep[2]   # h2b = [e3; e2]
        wepp[0:64, 1, j, 4 * j + 3] = wep[3]

    gw2p = np.zeros((128, 4, 8, 32), f)
    for g in range(4):
        for j in range(8):
            gw2p[32 * g : 32 * g + 32, g, j, 4 * j : 4 * j + 4] = gw2

    wtsb = np.concatenate(
        [w2p, wepp.reshape(128, 512), gw2p.reshape(128, 1024)], axis=1)
    assert wtsb.shape == (128, W_B), wtsb.shape
    return dict(wts=np.ascontiguousarray(wts),
                wtsb=np.ascontiguousarray(wtsb.astype(ml_dtypes.bfloat16)))


def kernel(**inputs):
    x = np.asarray(inputs["x"], dtype=np.float32)
    wk = {k: np.asarray(v, dtype=np.float32) for k, v in inputs.items()
          if k != "x"}
    packed = _pack_host(**wk)

    if "nc" not in _CACHE:
        _CACHE["nc"] = _build()
    nc = _CACHE["nc"]

    in_maps = []
    for c in range(NCORES):
        xc = x[c * BC : (c + 1) * BC]                 # (BC, 59)
        xt = np.zeros((64, BC), np.float32)
        xt[:IN] = xc.T
        xt[IN] = 1.0
        xi = np.ascontiguousarray(
            xt.reshape(64, SUP, 2, S).transpose(1, 2, 0, 3).reshape(SUP, 128, S)
        )
        m = {"x": xi}
        m.update(packed)
        in_maps.append(m)

    res = run_bass_kernel_spmd(nc, in_maps, core_ids=list(range(NCORES)))
    _CACHE["last"] = res
    outs = [r["out"].reshape(BC) for r in res.results]
    return np.concatenate(outs).reshape(B, 1).astype(np.float32)



# revision 7
# speedup vs baseline: 1.0324x; 1.0324x over previous
"""Trainium2 Bass kernel for nn_MixtureOfExperts (B=524288, IN=59, E=4, H=64).

Data-parallel over 8 cores (65536 rows each).  v2 design:

 - Host folds BN into weights/biases, collapses the embed head w3@wp -> wep
   (H->1 per expert), pre-transposes x into feature-major [feat, batch]
   fp16 layout (two independent 64-feature batch-halves on partition
   halves), and emits an fp8 (e4m3) hi/lo pair of x for the gating path.
 - Stage 1: fp16 matmuls, one per expert-pair (M=128 = 2 experts x H=64),
   biases folded via the ones-row of x.
 - Stage 2: fp16 block-diagonal matmuls (K=128 = 2 experts' h1 features,
   M=128 = 2 experts' h2) -- half the matmul count of per-expert K=64.
 - Gating hidden: fp8 DoubleRow matmuls (2 K-tiles: x8 and the scaled
   residual r8), 0.5 cycles/row; gw1 quantization noise only perturbs the
   softmax gates (~4e-3 final rel err).
 - preds / logits: tiny-N matmuls with h2 / g1 slices as the *stationary*
   operand and wep / gw2 as the moving operand -- cost is the output free
   size (2-4 columns) instead of a full 512-column pass.  Outputs land
   batch-major in one PSUM bank per q-group.
 - Tail (per q-group of 8 tiles): softmax-combine in batch-major layout
   with cheap strided DVE/Pool ops.
 - PSUM->SBUF evictions are spread across Pool/Act/DVE to balance engines.
"""

import numpy as np
import ml_dtypes

import concourse.bass as bass
import concourse.mybir as mybir
import concourse.tile as tile
from concourse import bacc
from concourse.bass_utils import run_bass_kernel_spmd

F32 = mybir.dt.float32
FP16 = mybir.dt.float16
FP8 = mybir.dt.float8e4
AF = mybir.ActivationFunctionType
ALU = mybir.AluOpType
DR = mybir.MatmulPerfMode.DoubleRow

B, IN, E, H, EMB, GH = 524288, 59, 4, 64, 32, 32
EPS = 1e-5
NCORES = 8
BC = B // NCORES          # 65536 rows per core
S = 8192                  # rows per batch-half per superstep
SUP = BC // (2 * S)       # 4 supersteps
NQ = S // (4 * 512)       # 4 q-groups per superstep
BT = 512

_CACHE = {}


def _build():
    nc = bacc.Bacc(trn_type="TRN2")
    x16_d = nc.dram_tensor("x16", (SUP, 128, S), FP16, kind="ExternalInput")
    x8_d = nc.dram_tensor("x8", (SUP, 128, 2 * S), FP8, kind="ExternalInput")
    w16_d = nc.dram_tensor("w16", (128, 692), FP16, kind="ExternalInput")
    w8_d = nc.dram_tensor("w8", (128, 1024), FP8, kind="ExternalInput")
    cst_d = nc.dram_tensor("cst", (128, 1024), F32, kind="ExternalInput")
    out_d = nc.dram_tensor("out", (SUP, 128, 128), F32, kind="ExternalOutput")

    with tile.TileContext(nc) as tc:
        with (
            tc.tile_pool(name="consts", bufs=1) as consts,
            tc.tile_pool(name="xp", bufs=2) as xp,
            tc.tile_pool(name="x8p", bufs=2) as x8p,
            tc.tile_pool(name="hs", bufs=2) as hs,
            tc.tile_pool(name="gs", bufs=2) as gs,
            tc.tile_pool(name="tl", bufs=2) as tl,
            tc.tile_pool(name="outp", bufs=2) as outp,
            tc.tile_pool(name="ph1", bufs=1, space="PSUM") as ph1,
            tc.tile_pool(name="ph2", bufs=1, space="PSUM") as ph2,
            tc.tile_pool(name="pgw", bufs=1, space="PSUM") as pgw,
            tc.tile_pool(name="plp", bufs=2, space="PSUM") as plp,
        ):
            w16_sb = consts.tile([128, 692], FP16)
            nc.sync.dma_start(out=w16_sb, in_=w16_d[:, :])
            w8_sb = consts.tile([128, 1024], FP8)
            nc.sync.dma_start(out=w8_sb, in_=w8_d[:, :])
            cst_sb = consts.tile([128, 1024], F32)
            nc.sync.dma_start(out=cst_sb, in_=cst_d[:, :])

            w1p = w16_sb[:, 0:256]         # stage1 lhsT: pair01 | pair23
            w2b = w16_sb[:, 256:512]       # stage2 block-diag: pair01 | pair23
            wep = w16_sb[:, 512:516]       # preds moving cols (4)
            gw2z = w16_sb[:, 516:532]      # logits moving cols, 4 zero-padded slots
            w8v = w8_sb.rearrange("p (g two m) -> p g two m", g=4, two=2)
            c2w = cst_sb[:, 0:1024]              # c2a | c2b wide bias
            biascol = w16_sb[:, 532:564]         # bias-mm moving cols
            e0row = w16_sb[:, 564:692]           # bias-mm lhsT (row0 = 1)

            for k in range(SUP):
                x16_sb = xp.tile([128, S], FP16, tag="x16")
                for ch in range(4):
                    cw = S // 4
                    nc.sync.dma_start(
                        out=x16_sb[:, ch * cw : (ch + 1) * cw],
                        in_=x16_d[k][:, ch * cw : (ch + 1) * cw])
                x8_sb = x8p.tile([128, 2 * S], FP8, tag="x8")
                for ch in range(4):
                    cw = 2 * S // 4
                    nc.sync.dma_start(
                        out=x8_sb[:, ch * cw : (ch + 1) * cw],
                        in_=x8_d[k][:, ch * cw : (ch + 1) * cw])
                x8v = x8_sb.rearrange("p (two s) -> p two s", two=2)
                out_acc = outp.tile([128, 128], F32, tag="oacc")

                for q in range(NQ):
                    # ---- gating: fp8 DoubleRow, 4 strip-slots per half
                    gw = pgw.tile([128, 2 * BT], F32, tag="gw")
                    for gi in range(4):
                        c0 = (4 * q + gi) * BT
                        for half in (0, 1):
                            base = 64 * half
                            nc.tensor.matmul(
                                out=gw[:, half * BT : half * BT + BT],
                                lhsT=w8v[base : base + 64, gi],
                                rhs=x8v[base : base + 64, :, c0 : c0 + BT],
                                start=(gi == 0), stop=(gi == 3),
                                perf_mode=DR, skip_group_check=True)
                    g1 = gs.tile([128, 2 * BT], FP16, tag="g1")
                    nc.scalar.activation(g1, gw, AF.Relu, scale=1.0 / 16.0)

                    PL = plp.tile([128, 256], F32, tag="pl")
                    for pi in range(4):
                        c0 = (4 * q + pi) * BT
                        for half in (0, 1):
                            tm = 2 * pi + half
                            base = 64 * half
                            xs = x16_sb[base : base + 64, c0 : c0 + BT]
                            # ---- stage 1 (fp16), c1/gb1 via ones-row
                            h1 = ph1.tile([128, 2 * BT], F32, tag="h1")
                            nc.tensor.matmul(
                                out=h1[:, 0:BT],
                                lhsT=w1p[base : base + 64, 0:128],
                                rhs=xs, start=True, stop=True,
                                skip_group_check=True)
                            nc.tensor.matmul(
                                out=h1[:, BT : 2 * BT],
                                lhsT=w1p[base : base + 64, 128:256],
                                rhs=xs, start=True, stop=True,
                                skip_group_check=True)
                            h1s = hs.tile([128, 2 * BT], FP16, tag="h1s")
                            nc.scalar.activation(h1s, h1, AF.Relu)
                            # ---- stage 2 (fp16 block-diag K=128)
                            h2 = ph2.tile([128, 2 * BT], F32, tag="h2")
                            nc.tensor.matmul(
                                out=h2[:, 0:BT], lhsT=w2b[:, 0:128],
                                rhs=h1s[:, 0:BT], start=True, stop=True,
                                skip_group_check=True)
                            nc.tensor.matmul(
                                out=h2[:, BT : 2 * BT], lhsT=w2b[:, 128:256],
                                rhs=h1s[:, BT : 2 * BT], start=True,
                                stop=True, skip_group_check=True)
                            z2 = hs.tile([128, 2 * BT], FP16, tag="z2")
                            nc.vector.tensor_add(z2, h2, c2w)
                            h2s = hs.tile([128, 2 * BT], FP16, tag="h2s")
                            nc.gpsimd.tensor_scalar(
                                h2s, z2, 0.0, None, ALU.max)
                            # ---- bias + preds + logits tiny matmuls
                            j0 = 32 * tm
                            nc.tensor.matmul(
                                out=PL[:, j0 : j0 + 32], lhsT=e0row,
                                rhs=biascol, start=True, stop=False,
                                skip_group_check=True)
                            for s in range(4):
                                sla = slice(128 * s, 128 * s + 128)
                                slb = slice(BT + 128 * s, BT + 128 * s + 128)
                                nc.tensor.matmul(
                                    out=PL[:, j0 + 4 * s : j0 + 4 * s + 2],
                                    lhsT=h2s[:, sla], rhs=wep[:, 0:2],
                                    start=False, stop=False,
                                    skip_group_check=True)
                                nc.tensor.matmul(
                                    out=PL[:, j0 + 4 * s + 2 : j0 + 4 * s + 4],
                                    lhsT=h2s[:, slb], rhs=wep[:, 2:4],
                                    start=False, stop=False,
                                    skip_group_check=True)
                                g1x = (g1[:, 0:BT] if half == 0
                                       else g1[:, BT : 2 * BT])
                                nc.tensor.matmul(
                                    out=PL[:, j0 + 16 + 4 * s :
                                           j0 + 16 + 4 * s + 4],
                                    lhsT=g1x[:, sla],
                                    rhs=gw2z[:, 4 * pi : 4 * pi + 4],
                                    start=False, stop=(s == 3),
                                    skip_group_check=True)

                    # ---- tail for this q-group (8 tiles, batch-major)
                    PLv = PL.rearrange("p (t j) -> p t j", t=8)
                    EL = tl.tile([128, 128], FP16, tag="el")
                    nc.scalar.activation(
                        EL.rearrange("p (t j) -> p t j", t=8),
                        PLv[:, :, 16:32], AF.Exp)
                    W = tl.tile([128, 128], FP16, tag="w")
                    nc.vector.tensor_mul(
                        W.rearrange("p (t j) -> p t j", t=8),
                        PLv[:, :, 0:16],
                        EL.rearrange("p (t j) -> p t j", t=8))
                    Wv = W.rearrange("p (n e) -> p n e", e=4)
                    ELv = EL.rearrange("p (n e) -> p n e", e=4)
                    n1 = tl.tile([128, 64], F32, tag="n1")
                    n1v = n1.rearrange("p (n e) -> p n e", e=2)
                    nc.gpsimd.tensor_add(n1v, Wv[:, :, 0:2], Wv[:, :, 2:4])
                    d1 = tl.tile([128, 64], F32, tag="d1")
                    d1v = d1.rearrange("p (n e) -> p n e", e=2)
                    nc.gpsimd.tensor_add(d1v, ELv[:, :, 0:2], ELv[:, :, 2:4])
                    num = tl.tile([128, 32], F32, tag="num")
                    nc.gpsimd.tensor_add(
                        num.rearrange("p (n e) -> p n e", e=1),
                        n1v[:, :, 0:1], n1v[:, :, 1:2])
                    den = tl.tile([128, 32], F32, tag="den")
                    nc.gpsimd.tensor_add(
                        den.rearrange("p (n e) -> p n e", e=1),
                        d1v[:, :, 0:1], d1v[:, :, 1:2])
                    rec = tl.tile([128, 32], F32, tag="rec")
                    nc.vector.reciprocal(rec, den)
                    nc.gpsimd.tensor_mul(
                        out_acc[:, 32 * q : 32 * q + 32], num, rec)

                nc.sync.dma_start(out=out_d[k], in_=out_acc)

    if not nc.is_finalized():
        nc.finalize()
    return nc


def _q8(a, scale):
    return np.asarray(a * scale, np.float32).astype(ml_dtypes.float8_e4m3)


def _pack_host(w1, b1, bn1_g, bn1_b, bn1_m, bn1_v, w2, b2, bn2_g, bn2_b,
               bn2_m, bn2_v, w3, b3, wp, bp, gw1, gb1, gw2, gb2):
    f = np.float32
    s1 = (bn1_g / np.sqrt(bn1_v + EPS)).astype(f)              # (E,H)
    w1e = (w1 * s1[:, None, :]).astype(f)                       # (E,IN,H)
    c1 = ((b1 - bn1_m) * s1 + bn1_b).astype(f)                  # (E,H)
    s2 = (bn2_g / np.sqrt(bn2_v + EPS)).astype(f)
    w2e = (w2 * s2[:, None, :]).astype(f)                       # (E,H,H)
    c2 = ((b2 - bn2_m) * s2 + bn2_b).astype(f)                  # (E,H)
    wepv = np.einsum("ehm,em->eh", w3, wp).astype(f)            # (E,H)
    bep = (np.einsum("em,em->e", b3, wp) + bp).astype(f)        # (E,)

    # ---- fp16 block [128, 692]
    w16d = np.zeros((128, 692), f)
    for pr in range(2):                       # stage1 lhsT, expert pairs
        for j in range(2):
            e = 2 * pr + j
            for half in range(2):
                r0 = 64 * half
                w16d[r0 : r0 + IN, 128 * pr + 64 * j : 128 * pr + 64 * j + 64] = w1e[e]
                w16d[r0 + IN, 128 * pr + 64 * j : 128 * pr + 64 * j + 64] = c1[e]
    for pr in range(2):                       # stage2 block-diag lhsT
        e0, e1 = 2 * pr, 2 * pr + 1
        blk = np.zeros((128, 128), f)
        blk[0:64, 0:64] = w2e[e0]
        blk[64:128, 64:128] = w2e[e1]
        w16d[:, 256 + 128 * pr : 384 + 128 * pr] = blk
    wepp = np.zeros((128, 4), f)              # preds moving cols
    wepp[0:64, 0] = wepv[0]
    wepp[64:128, 1] = wepv[1]
    wepp[0:64, 2] = wepv[2]
    wepp[64:128, 3] = wepv[3]
    w16d[:, 512:516] = wepp
    for g in range(4):                # logits moving cols, zero-padded slots
        w16d[32 * g : 32 * g + 32, 516 + 4 * g : 520 + 4 * g] = gw2
    pat = np.zeros(32, f)                     # bias-mm moving cols (row 0)
    for s in range(4):
        for e in range(4):
            pat[4 * s + e] = bep[e]
            pat[16 + 4 * s + e] = gb2[e]
    w16d[0, 532:564] = pat
    w16d[0, 564:692] = 1.0                    # e0row lhsT: row0 = ones
    w16d = w16d.astype(np.float16)

    # ---- fp8 gate lhsT [128, 1024] = [p, slot g, ktile, 128]
    w8 = np.zeros((64, 4, 2, 128), np.float32)
    for g in range(4):
        w8[:IN, g, 0, 32 * g : 32 * g + 32] = 16.0 * gw1
        w8[:IN, g, 1, 32 * g : 32 * g + 32] = 2.0 * gw1
        w8[IN, g, 0, 32 * g : 32 * g + 32] = 16.0 * gb1
    w8 = np.concatenate([w8, w8], axis=0).reshape(128, 1024)
    w8 = w8.astype(ml_dtypes.float8_e4m3)

    # ---- f32 consts [128, 1024]: c2 wide bias (pair01 | pair23)
    cst = np.zeros((128, 1024), f)
    cst[:, 0:512] = np.concatenate([c2[0], c2[1]])[:, None]
    cst[:, 512:1024] = np.concatenate([c2[2], c2[3]])[:, None]
    return dict(w16=np.ascontiguousarray(w16d),
                w8=np.ascontiguousarray(w8),
                cst=np.ascontiguousarray(cst))


def _prep_x_core(xc):
    """xc: (BC, 59) f32 -> x16 [SUP,128,S] fp16, x8 [SUP,128,2S] fp8."""
    xt = np.zeros((64, BC), np.float32)
    xt[:IN] = xc.T
    xt[IN] = 1.0
    # [64, BC] -> [SUP, 2(half), 64, S] -> [SUP, 128, S]
    xq = xt.reshape(64, SUP, 2, S).transpose(1, 2, 0, 3).reshape(SUP, 128, S)
    x16 = np.ascontiguousarray(xq).astype(np.float16)
    x8 = xq.astype(ml_dtypes.float8_e4m3)
    r = xq - x8.astype(np.float32)
    r8 = _q8(r, 8.0)
    # kill the residual of the ones-row (row 59 within each 64-block)
    r8.reshape(SUP, 2, 64, S)[:, :, IN:, :] = 0
    x8c = np.concatenate([x8, r8], axis=2)          # [SUP, 128, 2S]
    return x16, np.ascontiguousarray(x8c)


def _unpack_out(o):
    """o: [SUP, 128, 128] f32 -> (BC,) f32."""
    # col = 32q + 4tm + s ; tm = 2*pi + half
    v = o.reshape(SUP, 128, 4, 4, 2, 4)       # k, p, q, pi, half, s
    v = v.transpose(0, 4, 2, 3, 5, 1)         # k, half, q, pi, s, p
    return v.reshape(BC)


def kernel(**inputs):
    x = np.asarray(inputs["x"], dtype=np.float32)
    wk = {kk: np.asarray(v, dtype=np.float32) for kk, v in inputs.items()
          if kk != "x"}
    packed = _pack_host(**wk)

    if "nc" not in _CACHE:
        _CACHE["nc"] = _build()
    nc = _CACHE["nc"]

    in_maps = []
    for c in range(NCORES):
        x16, x8 = _prep_x_core(x[c * BC : (c + 1) * BC])
        m = {"x16": x16, "x8": x8}
        m.update(packed)
        in_maps.append(m)

    res = run_bass_kernel_spmd(nc, in_maps, core_ids=list(range(NCORES)))
    _CACHE["last"] = res
    outs = [_unpack_out(r["out"]) for r in res.results]
    return np.concatenate(outs).reshape(B, 1).astype(np.float32)


# revision 11
# speedup vs baseline: 1.4127x; 1.3684x over previous
"""Trainium2 Bass kernel for nn_MixtureOfExperts (B=524288, IN=59, E=4, H=64).

Data-parallel over 8 cores (65536 rows each).  v2 design:

 - Host folds BN into weights/biases, collapses the embed head w3@wp -> wep
   (H->1 per expert), pre-transposes x into feature-major [feat, batch]
   fp16 layout (two independent 64-feature batch-halves on partition
   halves), and emits an fp8 (e4m3) hi/lo pair of x for the gating path.
 - Stage 1: fp16 matmuls, one per expert-pair (M=128 = 2 experts x H=64),
   biases folded via the ones-row of x.
 - Stage 2: fp16 block-diagonal matmuls (K=128 = 2 experts' h1 features,
   M=128 = 2 experts' h2) -- half the matmul count of per-expert K=64.
 - Gating hidden: fp8 DoubleRow matmuls (2 K-tiles: x8 and the scaled
   residual r8), 0.5 cycles/row; gw1 quantization noise only perturbs the
   softmax gates (~4e-3 final rel err).
 - preds / logits: tiny-N matmuls with h2 / g1 slices as the *stationary*
   operand and wep / gw2 as the moving operand -- cost is the output free
   size (2-4 columns) instead of a full 512-column pass.  Outputs land
   batch-major in one PSUM bank per q-group.
 - Tail (per q-group of 8 tiles): softmax-combine in batch-major layout
   with cheap strided DVE/Pool ops.
 - PSUM->SBUF evictions are spread across Pool/Act/DVE to balance engines.
"""

import numpy as np
import ml_dtypes

import concourse.bass as bass
import concourse.mybir as mybir
import concourse.tile as tile
from concourse import bacc
from concourse.bass_utils import run_bass_kernel_spmd

F32 = mybir.dt.float32
FP16 = mybir.dt.float16
FP8 = mybir.dt.float8e4
AF = mybir.ActivationFunctionType
ALU = mybir.AluOpType
DR = mybir.MatmulPerfMode.DoubleRow

B, IN, E, H, EMB, GH = 524288, 59, 4, 64, 32, 32
EPS = 1e-5
NCORES = 8
BC = B // NCORES          # 65536 rows per core
S = 8192                  # rows per batch-half per superstep
SUP = BC // (2 * S)       # 4 supersteps
NQ = S // (4 * 512)       # 4 q-groups per superstep
BT = 512

_CACHE = {}


def _build():
    nc = bacc.Bacc(trn_type="TRN2")
    x16_d = nc.dram_tensor("x16", (SUP, 128, S), FP16, kind="ExternalInput")
    x8_d = nc.dram_tensor("x8", (SUP, 128, 2 * S), FP8, kind="ExternalInput")
    w16_d = nc.dram_tensor("w16", (128, 692), FP16, kind="ExternalInput")
    w8_d = nc.dram_tensor("w8", (128, 1024), FP8, kind="ExternalInput")
    cst_d = nc.dram_tensor("cst", (128, 1024), F32, kind="ExternalInput")
    out_d = nc.dram_tensor("out", (SUP, 128, 128), F32, kind="ExternalOutput")

    with tile.TileContext(nc) as tc:
        with (
            tc.tile_pool(name="consts", bufs=1) as consts,
            tc.tile_pool(name="xp", bufs=2) as xp,
            tc.tile_pool(name="x8p", bufs=2) as x8p,
            tc.tile_pool(name="hs", bufs=3) as hs,
            tc.tile_pool(name="gs", bufs=2) as gs,
            tc.tile_pool(name="tl", bufs=2) as tl,
            tc.tile_pool(name="outp", bufs=2) as outp,
            tc.tile_pool(name="pwork", bufs=3, space="PSUM") as pwork,
            tc.tile_pool(name="pgate", bufs=1, space="PSUM") as pgate,
            tc.tile_pool(name="plp", bufs=1, space="PSUM") as plp,
        ):
            w16_sb = consts.tile([128, 692], FP16)
            nc.sync.dma_start(out=w16_sb, in_=w16_d[:, :])
            w8_sb = consts.tile([128, 1024], FP8)
            nc.sync.dma_start(out=w8_sb, in_=w8_d[:, :])
            cst_sb = consts.tile([128, 1024], F32)
            nc.sync.dma_start(out=cst_sb, in_=cst_d[:, :])

            w1p = w16_sb[:, 0:256]         # stage1 lhsT: pair01 | pair23
            w2b = w16_sb[:, 256:512]       # stage2 block-diag: pair01 | pair23
            wep = w16_sb[:, 512:516]       # preds moving cols (4)
            gw2z = w16_sb[:, 516:532]      # logits moving cols, zero-padded
            biascol = w16_sb[:, 532:564]   # bias-mm moving cols
            e0row = w16_sb[:, 564:692]     # bias-mm lhsT (row0 = 1)
            w8v = w8_sb.rearrange("p (g two m) -> p g two m", g=4, two=2)
            c2aw = cst_sb[:, 0:1]          # stage2 bias, pair01
            c2bw = cst_sb[:, 512:513]      # stage2 bias, pair23

            GPQ = 4                       # 2-tile groups per q-group
            TOT = SUP * NQ * GPQ          # total groups
            grp = {}                      # live pipeline state per group
            perq = {}                     # (k, q) -> {PL, g1a, g1b}
            perk = {}                     # k -> {x16, x8v, oacc}
            GT = [[(0, 0), (0, 1)], [(0, 2), (0, 3)],
                  [(1, 0), (1, 1)], [(1, 2), (1, 3)]]

            def load_ss(k):
                x16_sb = xp.tile([128, S], FP16, tag="x16")
                for ch in range(4):
                    cw = S // 4
                    nc.sync.dma_start(
                        out=x16_sb[:, ch * cw : (ch + 1) * cw],
                        in_=x16_d[k][:, ch * cw : (ch + 1) * cw])
                x8_sb = x8p.tile([128, 2 * S], FP8, tag="x8")
                for ch in range(4):
                    cw = 2 * S // 4
                    nc.sync.dma_start(
                        out=x8_sb[:, ch * cw : (ch + 1) * cw],
                        in_=x8_d[k][:, ch * cw : (ch + 1) * cw])
                perk[k] = {"x16": x16_sb,
                           "x8v": x8_sb.rearrange("p (two s) -> p two s",
                                                  two=2)}

            def gate_round(k, q, half):
                gw = pgate.tile([128, BT], F32, tag="gw")
                base = 64 * half
                for gi in range(4):
                    c0 = (4 * q + gi) * BT
                    nc.tensor.matmul(
                        out=gw,
                        lhsT=w8v[base : base + 64, gi],
                        rhs=perk[k]["x8v"][base : base + 64, :,
                                           c0 : c0 + BT],
                        start=(gi == 0), stop=(gi == 3),
                        perf_mode=DR, skip_group_check=True)
                g1 = gs.tile([128, BT], FP16, tag="g1")
                nc.scalar.activation(g1, gw, AF.Relu, scale=1.0 / 16.0)
                perq[(k, q)]["g1a" if half == 0 else "g1b"] = g1

            def emit_s1(g):
                k, q, j = g // (NQ * GPQ), (g // GPQ) % NQ, g % GPQ
                st = {"k": k, "q": q, "tiles": GT[j], "h1s": []}
                grp[g] = st
                for half, pi in GT[j]:
                    c0 = (4 * q + pi) * BT
                    base = 64 * half
                    xs = perk[k]["x16"][base : base + 64, c0 : c0 + BT]
                    h1 = pwork.tile([128, 2 * BT], F32, tag="w")
                    nc.tensor.matmul(
                        out=h1[:, 0:BT],
                        lhsT=w1p[base : base + 64, 0:128],
                        rhs=xs, start=True, stop=True,
                        skip_group_check=True)
                    nc.tensor.matmul(
                        out=h1[:, BT : 2 * BT],
                        lhsT=w1p[base : base + 64, 128:256],
                        rhs=xs, start=True, stop=True,
                        skip_group_check=True)
                    h1s = hs.tile([128, 2 * BT], FP16, tag="h1s")
                    nc.scalar.activation(h1s, h1, AF.Relu)
                    st["h1s"].append(h1s)

            def emit_s2(g):
                st = grp[g]
                h2A = pwork.tile([128, 2 * BT], F32, tag="w")
                h2B = pwork.tile([128, 2 * BT], F32, tag="w")
                for i in range(2):
                    cc = i * BT
                    nc.tensor.matmul(
                        out=h2A[:, cc : cc + BT], lhsT=w2b[:, 0:128],
                        rhs=st["h1s"][i][:, 0:BT], start=True, stop=True,
                        skip_group_check=True)
                    nc.tensor.matmul(
                        out=h2B[:, cc : cc + BT], lhsT=w2b[:, 128:256],
                        rhs=st["h1s"][i][:, BT : 2 * BT], start=True,
                        stop=True, skip_group_check=True)
                sa = hs.tile([128, 2 * BT], FP16, tag="h2sa")
                nc.vector.tensor_scalar(
                    sa, h2A, c2aw, 0.0, ALU.add, ALU.max)
                sb = hs.tile([128, 2 * BT], FP16, tag="h2sb")
                nc.vector.tensor_scalar(
                    sb, h2B, c2bw, 0.0, ALU.add, ALU.max)
                st["sa"], st["sb"] = sa, sb

            def emit_tinies(g):
                st = grp[g]
                k, q = st["k"], st["q"]
                pq = perq[(k, q)]
                if "PL" not in pq:
                    pq["PL"] = plp.tile([128, 256], F32, tag="pl", name="PL")
                PL = pq["PL"]
                for i, (half, pi) in enumerate(st["tiles"]):
                    tm = 4 * half + pi
                    j0 = 32 * tm
                    cc = i * BT
                    sa, sb = st["sa"], st["sb"]
                    g1x = pq["g1a"] if half == 0 else pq["g1b"]
                    nc.tensor.matmul(
                        out=PL[:, j0 : j0 + 32], lhsT=e0row,
                        rhs=biascol, start=True, stop=False,
                        skip_group_check=True)
                    for s in range(4):
                        sl = slice(cc + 128 * s, cc + 128 * s + 128)
                        sg = slice(128 * s, 128 * s + 128)
                        nc.tensor.matmul(
                            out=PL[:, j0 + 4 * s : j0 + 4 * s + 2],
                            lhsT=sa[:, sl], rhs=wep[:, 0:2],
                            start=False, stop=False,
                            skip_group_check=True)
                        nc.tensor.matmul(
                            out=PL[:, j0 + 4 * s + 2 : j0 + 4 * s + 4],
                            lhsT=sb[:, sl], rhs=wep[:, 2:4],
                            start=False, stop=False,
                            skip_group_check=True)
                        nc.tensor.matmul(
                            out=PL[:, j0 + 16 + 4 * s : j0 + 20 + 4 * s],
                            lhsT=g1x[:, sg],
                            rhs=gw2z[:, 4 * pi : 4 * pi + 4],
                            start=False, stop=(s == 3),
                            skip_group_check=True)
                del grp[g]

            def emit_tail(k, q):
                PL = perq[(k, q)]["PL"]
                PLv = PL.rearrange("p (t j) -> p t j", t=8)
                EL = tl.tile([128, 128], FP16, tag="el")
                nc.scalar.activation(
                    EL.rearrange("p (t j) -> p t j", t=8),
                    PLv[:, :, 16:32], AF.Exp)
                W = tl.tile([128, 128], FP16, tag="w")
                nc.vector.tensor_mul(
                    W.rearrange("p (t j) -> p t j", t=8),
                    PLv[:, :, 0:16],
                    EL.rearrange("p (t j) -> p t j", t=8))
                Wv = W.rearrange("p (n e) -> p n e", e=4)
                ELv = EL.rearrange("p (n e) -> p n e", e=4)
                n1 = tl.tile([128, 64], F32, tag="n1")
                n1v = n1.rearrange("p (n e) -> p n e", e=2)
                nc.gpsimd.tensor_add(n1v, Wv[:, :, 0:2], Wv[:, :, 2:4])
                d1 = tl.tile([128, 64], F32, tag="d1")
                d1v = d1.rearrange("p (n e) -> p n e", e=2)
                nc.gpsimd.tensor_add(d1v, ELv[:, :, 0:2], ELv[:, :, 2:4])
                num = tl.tile([128, 32], F32, tag="num")
                nc.gpsimd.tensor_add(
                    num.rearrange("p (n e) -> p n e", e=1),
                    n1v[:, :, 0:1], n1v[:, :, 1:2])
                den = tl.tile([128, 32], F32, tag="den")
                nc.gpsimd.tensor_add(
                    den.rearrange("p (n e) -> p n e", e=1),
                    d1v[:, :, 0:1], d1v[:, :, 1:2])
                rec = tl.tile([128, 32], F32, tag="rec")
                nc.vector.reciprocal(rec, den)
                if "oacc" not in perk[k]:
                    perk[k]["oacc"] = outp.tile([128, 128], F32, tag="oacc", name="oacc")
                nc.gpsimd.tensor_mul(
                    perk[k]["oacc"][:, 32 * q : 32 * q + 32], num, rec)
                if q == NQ - 1:
                    nc.sync.dma_start(out=out_d[k], in_=perk[k]["oacc"])

            for g in range(TOT + 2):
                if g < TOT:
                    k = g // (NQ * GPQ)
                    q = (g // GPQ) % NQ
                    j = g % GPQ
                    if j == 0:
                        if q == 0:
                            load_ss(k)
                        perq[(k, q)] = {}
                        gate_round(k, q, 0)
                    if j == 2:
                        gate_round(k, q, 1)
                    emit_s1(g)
                if 1 <= g < TOT + 1:
                    emit_s2(g - 1)
                if 2 <= g < TOT + 2:
                    gg = g - 2
                    emit_tinies(gg)
                    if gg % GPQ == GPQ - 1:
                        emit_tail(gg // (NQ * GPQ), (gg // GPQ) % NQ)

    if not nc.is_finalized():
        nc.finalize()
    return nc


def _q8(a, scale):
    return np.asarray(a * scale, np.float32).astype(ml_dtypes.float8_e4m3)


def _pack_host(w1, b1, bn1_g, bn1_b, bn1_m, bn1_v, w2, b2, bn2_g, bn2_b,
               bn2_m, bn2_v, w3, b3, wp, bp, gw1, gb1, gw2, gb2):
    f = np.float32
    s1 = (bn1_g / np.sqrt(bn1_v + EPS)).astype(f)              # (E,H)
    w1e = (w1 * s1[:, None, :]).astype(f)                       # (E,IN,H)
    c1 = ((b1 - bn1_m) * s1 + bn1_b).astype(f)                  # (E,H)
    s2 = (bn2_g / np.sqrt(bn2_v + EPS)).astype(f)
    w2e = (w2 * s2[:, None, :]).astype(f)                       # (E,H,H)
    c2 = ((b2 - bn2_m) * s2 + bn2_b).astype(f)                  # (E,H)
    wepv = np.einsum("ehm,em->eh", w3, wp).astype(f)            # (E,H)
    bep = (np.einsum("em,em->e", b3, wp) + bp).astype(f)        # (E,)

    # ---- fp16 block [128, 692]
    w16d = np.zeros((128, 692), f)
    for pr in range(2):                       # stage1 lhsT, expert pairs
        for j in range(2):
            e = 2 * pr + j
            for half in range(2):
                r0 = 64 * half
                w16d[r0 : r0 + IN, 128 * pr + 64 * j : 128 * pr + 64 * j + 64] = w1e[e]
                w16d[r0 + IN, 128 * pr + 64 * j : 128 * pr + 64 * j + 64] = c1[e]
    for pr in range(2):                       # stage2 block-diag lhsT
        e0, e1 = 2 * pr, 2 * pr + 1
        blk = np.zeros((128, 128), f)
        blk[0:64, 0:64] = w2e[e0]
        blk[64:128, 64:128] = w2e[e1]
        w16d[:, 256 + 128 * pr : 384 + 128 * pr] = blk
    wepp = np.zeros((128, 4), f)              # preds moving cols
    wepp[0:64, 0] = wepv[0]
    wepp[64:128, 1] = wepv[1]
    wepp[0:64, 2] = wepv[2]
    wepp[64:128, 3] = wepv[3]
    w16d[:, 512:516] = wepp
    for g in range(4):                # logits moving cols, zero-padded slots
        w16d[32 * g : 32 * g + 32, 516 + 4 * g : 520 + 4 * g] = gw2
    pat = np.zeros(32, f)                     # bias-mm moving cols (row 0)
    for s in range(4):
        for e in range(4):
            pat[4 * s + e] = bep[e]
            pat[16 + 4 * s + e] = gb2[e]
    w16d[0, 532:564] = pat
    w16d[0, 564:692] = 1.0                    # e0row lhsT: row0 = ones
    w16d = w16d.astype(np.float16)

    # ---- fp8 gate lhsT [128, 1024] = [p, slot g, ktile, 128]
    w8 = np.zeros((64, 4, 2, 128), np.float32)
    for g in range(4):
        w8[:IN, g, 0, 32 * g : 32 * g + 32] = 16.0 * gw1
        w8[:IN, g, 1, 32 * g : 32 * g + 32] = 2.0 * gw1
        w8[IN, g, 0, 32 * g : 32 * g + 32] = 16.0 * gb1
    w8 = np.concatenate([w8, w8], axis=0).reshape(128, 1024)
    w8 = w8.astype(ml_dtypes.float8_e4m3)

    # ---- f32 consts [128, 1024]: c2 wide bias (pair01 | pair23)
    cst = np.zeros((128, 1024), f)
    cst[:, 0:512] = np.concatenate([c2[0], c2[1]])[:, None]
    cst[:, 512:1024] = np.concatenate([c2[2], c2[3]])[:, None]
    return dict(w16=np.ascontiguousarray(w16d),
                w8=np.ascontiguousarray(w8),
                cst=np.ascontiguousarray(cst))


def _prep_x_core(xc):
    """xc: (BC, 59) f32 -> x16 [SUP,128,S] fp16, x8 [SUP,128,2S] fp8."""
    xt = np.zeros((64, BC), np.float32)
    xt[:IN] = xc.T
    xt[IN] = 1.0
    # [64, BC] -> [SUP, 2(half), 64, S] -> [SUP, 128, S]
    xq = xt.reshape(64, SUP, 2, S).transpose(1, 2, 0, 3).reshape(SUP, 128, S)
    x16 = np.ascontiguousarray(xq).astype(np.float16)
    x8 = xq.astype(ml_dtypes.float8_e4m3)
    r = xq - x8.astype(np.float32)
    r8 = _q8(r, 8.0)
    # kill the residual of the ones-row (row 59 within each 64-block)
    r8.reshape(SUP, 2, 64, S)[:, :, IN:, :] = 0
    x8c = np.concatenate([x8, r8], axis=2)          # [SUP, 128, 2S]
    return x16, np.ascontiguousarray(x8c)


def _unpack_out(o):
    """o: [SUP, 128, 128] f32 -> (BC,) f32."""
    # col = 32q + 4tm + s ; tm = 4*half + pi
    v = o.reshape(SUP, 128, 4, 2, 4, 4)       # k, p, q, half, pi, s
    v = v.transpose(0, 3, 2, 4, 5, 1)         # k, half, q, pi, s, p
    return v.reshape(BC)


def kernel(**inputs):
    x = np.asarray(inputs["x"], dtype=np.float32)
    wk = {kk: np.asarray(v, dtype=np.float32) for kk, v in inputs.items()
          if kk != "x"}
    packed = _pack_host(**wk)

    if "nc" not in _CACHE:
        _CACHE["nc"] = _build()
    nc = _CACHE["nc"]

    in_maps = []
    for c in range(NCORES):
        x16, x8 = _prep_x_core(x[c * BC : (c + 1) * BC])
        m = {"x16": x16, "x8": x8}
        m.update(packed)
        in_maps.append(m)

    res = run_bass_kernel_spmd(nc, in_maps, core_ids=list(range(NCORES)))
    _CACHE["last"] = res
    outs = [_unpack_out(r["out"]) for r in res.results]
    return np.concatenate(outs).reshape(B, 1).astype(np.float32)
